# revision 46
# baseline (speedup 1.0000x reference)
"""GQA attention + RoPE + causal softmax + output projection on 8 TRN2 cores.

Sharding: tensor-parallel over heads. Core i owns q-heads [4i, 4i+4) and
kv-head i (GQA group size 4 == HQ/8, HK/8 = 1).

fp8 strategy (keeps rel err ~1%, well under the 2e-2 gate):
  - Q and K paths run PURE fp8e4 (x, wq, wk, and the rope'd Q^T/K^T all fp8):
    score errors are ~5% of |s| with |s| ~ 4e-3, so p = exp(s) moves by
    ~2e-4 absolute -- diluted to ~0.03% on the output by the softmax.
  - V path and the output projection carry first-order residual corrections:
    x = x8 + xr8, wv = wv8 + wvr8, attn = ao8 + aor8, wo = wo8 + wor8 (all
    fp8 pairs; the resid*resid cross term is dropped, ~0.4% second order).
  - All fp8 matmuls use MatmulPerfMode.DoubleRow (2 contraction subtiles per
    instruction at 0.5 cycles/output-column = 4x bf16 throughput). The
    HD=128 score contraction is split as [64 partitions x 2 subtiles].
  - P*V stays bf16 (p values cluster at 1.0; fp8 would quantize away the
    softmax signal).

Scale bookkeeping (powers of two, folded into existing constants):
  x8,w8 carry 2^7 each -> projection PSUM = 2^14 * true.
  cos2/sin2 carry 2^-8   -> Q^T/K^T fp8 = 2^6 * true; score PSUM = 2^12 * s.
  exp scale = 2^-12/sqrt(HD).  V stays scaled: vts = 2^14 * v.
  ones_mat = 16 = 2^(14-10)  -> ao = 2^10 * attn (good fp8 range).
  out PSUM = 2^(10+7) * true -> final ACT copy uses scale 2^-17.

Softmax denominator: DVE accumulates exp chunks into two bf16 accumulators
(even/odd chunks, so the serial add chain keeps up with the PE), then two
ones-matmuls fold the partition sum + broadcast + 2^4 scale in one step.

Collectives: two AllGathers over sequence halves (all 4 heads + resid rows
per half), issued after qb=1 and qb=3 of the qb-outer attention loop; the
output projection consumes half 0 while half 1 is still being gathered.
"""

import os

import numpy as np
import ml_dtypes

import concourse.bass as bass
import concourse.mybir as mybir
import concourse.tile as tile
from concourse import bacc
from concourse.bass_utils import run_bass_kernel_spmd

# Problem dims (hardcoded per contract)
B, S, D = 1, 2048, 4096
HQ, HK, HD = 32, 8, 128
NCORES = 8
HQL = HQ // NCORES          # 4 local q heads
SB = 512                    # seq block (matmul moving free dim)
NB = S // SB                # 4 seq blocks
NPAIR = D // 256            # 16 DoubleRow contraction pairs for D
SCALE = 1.0 / float(np.sqrt(HD))
EXP_SCALE = SCALE / 4096.0  # scores PSUM carries 2^12
S2 = S // 2                 # gather half width

F32 = mybir.dt.float32
BF16 = mybir.dt.bfloat16
FP8 = mybir.dt.float8e4
DR = mybir.MatmulPerfMode.DoubleRow
DEBUG_DUMPS = os.environ.get("BASSDBG", "") == "1"

# stream_shuffle mask: swap adjacent pairs within each 32-partition quadrant
SWAP_MASK = [(i ^ 1) for i in range(32)]


def _build_nc():
    nc = bacc.Bacc(
        "TRN2", target_bir_lowering=False, debug=False, num_devices=NCORES
    )

    io = {}
    io["x8"] = nc.dram_tensor("x8", [D, S], FP8, kind="ExternalInput")
    io["xr8"] = nc.dram_tensor("xr8", [D, S], FP8, kind="ExternalInput")
    io["wq8"] = nc.dram_tensor("wq8", [D, HQL * HD], FP8, kind="ExternalInput")
    io["wk8"] = nc.dram_tensor("wk8", [D, HD], FP8, kind="ExternalInput")
    io["wv8"] = nc.dram_tensor("wv8", [D, HD], FP8, kind="ExternalInput")
    io["wvr8"] = nc.dram_tensor("wvr8", [D, HD], FP8, kind="ExternalInput")
    io["wo"] = nc.dram_tensor("wo", [D, HQL * HD], BF16, kind="ExternalInput")
    io["cos2"] = nc.dram_tensor("cos2", [HD, S], BF16, kind="ExternalInput")
    io["sin2"] = nc.dram_tensor("sin2", [HD, S], BF16, kind="ExternalInput")
    # causal mask as a matmul: T8 lower-inclusive triangle (stationary) and
    # per-td moving panels R with -1e6 markers; T8.T @ R[td] adds -1e6 to
    # every (p, c) with c < 128*td + p, exactly the invalid region.
    io["mskT"] = nc.dram_tensor("mskT", [128, 128], BF16, kind="ExternalInput")
    io["mskR"] = nc.dram_tensor("mskR", [128, NB, SB], BF16, kind="ExternalInput")
    io["ident"] = nc.dram_tensor("ident", [128, 128], BF16, kind="ExternalInput")
    io["outT"] = nc.dram_tensor("outT", [HQL * HD, S], F32, kind="ExternalOutput")
    if DEBUG_DUMPS:
        io["dbg_qt"] = nc.dram_tensor(
            "dbg_qt", [128, NB, HQL, SB], FP8, kind="ExternalOutput"
        )
        io["dbg_kt"] = nc.dram_tensor(
            "dbg_kt", [128, NB, SB], FP8, kind="ExternalOutput"
        )
        io["dbg_vs"] = nc.dram_tensor(
            "dbg_vs", [128, NB, SB // 128, HD], BF16, kind="ExternalOutput"
        )
        io["dbg_loc"] = nc.dram_tensor(
            "dbg_loc", [HQL * HD, SB], BF16, kind="ExternalOutput"
        )

    with tile.TileContext(nc) as tc:
        _body(tc, io)
    nc.compile()
    return nc


def _body(tc, io):
    nc = tc.nc
    from contextlib import ExitStack

    ctx = ExitStack()
    with ctx:
        consts = ctx.enter_context(tc.tile_pool(name="consts", bufs=1))
        qkv = ctx.enter_context(tc.tile_pool(name="qkv", bufs=1))
        dram = ctx.enter_context(tc.tile_pool(name="dram", bufs=1, space="DRAM"))

        cos2 = consts.tile([HD, S], BF16)
        sin2 = consts.tile([HD, S], BF16)
        ident = consts.tile([128, 128], BF16)
        mskT = consts.tile([128, 128], BF16)
        mskR = consts.tile([128, NB, SB], BF16)
        # ones * 2^14: the denominator matmul folds partition-sum, broadcast
        # and the 2^14 V-path descale in one shot -> ao lands at true scale
        ones_mat = consts.tile([128, 128], BF16)
        nc.vector.memset(ones_mat, 16384.0)

        # persistent per-core tensors, split per s-block for fine-grained deps
        qt8f = [
            qkv.tile([128, HQL, SB], FP8, name=f"qt8f{sb}") for sb in range(NB)
        ]
        kt8f = [qkv.tile([128, SB], FP8, name=f"kt8f{sb}") for sb in range(NB)]
        # [64, 2, ...] split layouts for DoubleRow score matmuls
        qt8 = [
            qkv.tile([64, 2, HQL, SB], FP8, name=f"qt8_{sb}") for sb in range(NB)
        ]
        kt8 = [qkv.tile([64, 2, SB], FP8, name=f"kt8_{sb}") for sb in range(NB)]
        vs_sb = [
            qkv.tile([128, SB // 128, HD], BF16, name=f"vs{sb}") for sb in range(NB)
        ]

        # quarter-sequence bounce + gather buffers (bf16, 4 heads per qb
        # block); gathered row 128*c = contraction chunk c = q-head 4i+j
        attn_loc = [
            dram.tile([HQL * HD, SB], BF16, name=f"attn_loc{g2}")
            for g2 in range(NB)
        ]
        attn_g = [
            dram.tile(
                [NCORES * HQL * HD, SB],
                BF16,
                name=f"attn_g{g2}",
                addr_space="Shared",
            )
            for g2 in range(NB)
        ]

        # ================= Stage A: projections + RoPE =================
        with ctx_pools(tc) as (wpool, xpool, rpool, psA):
            # warm-up DoubleRow (discarded): the first dual-fp8 ldweights in a
            # program mis-executes (partial-NaN psum); absorb it on zeros.
            dmy_l = wpool.tile([128, 2, 128], FP8, name="dmy_l")
            dmy_r = wpool.tile([128, 2, 8], FP8, name="dmy_r")
            nc.vector.memset(dmy_l, 0.0)
            nc.vector.memset(dmy_r, 0.0)
            ps_warm = psA.tile([128, 8], F32, name="ps_warm", tag="psvt", bufs=2)
            nc.tensor.matmul(
                ps_warm, lhsT=dmy_l, rhs=dmy_r, start=True, stop=True, perf_mode=DR
            )

            wq_sb = wpool.tile([128, 2 * NPAIR, HQL * HD], FP8)
            wk_sb = wpool.tile([128, 2 * NPAIR, HD], FP8)
            wv_sb = wpool.tile([128, 2 * NPAIR, HD], FP8)
            wvr_sb = wpool.tile([128, 2 * NPAIR, HD], FP8)
            # first pair as fine slices so the PE can start ASAP
            for t in range(HQL):
                nc.gpsimd.dma_start(
                    out=wq_sb[:, 0:2, t * 128 : (t + 1) * 128],
                    in_=io["wq8"][0:256, t * 128 : (t + 1) * 128].rearrange(
                        "(c p) n -> p c n", p=128
                    ),
                )
            nc.gpsimd.dma_start(
                out=wk_sb[:, 0:2, :],
                in_=io["wk8"][0:256, :].rearrange("(c p) n -> p c n", p=128),
            )
            nc.gpsimd.dma_start(
                out=wv_sb[:, 0:2, :],
                in_=io["wv8"][0:256, :].rearrange("(c p) n -> p c n", p=128),
            )
            nc.gpsimd.dma_start(
                out=wvr_sb[:, 0:2, :],
                in_=io["wvr8"][0:256, :].rearrange("(c p) n -> p c n", p=128),
            )
            # pair 1 (the c4 bulk loop below starts at chunk 4)
            for w_sb, nm in (
                (wq_sb, "wq8"),
                (wk_sb, "wk8"),
                (wv_sb, "wv8"),
                (wvr_sb, "wvr8"),
            ):
                nc.gpsimd.dma_start(
                    out=w_sb[:, 2:4, :],
                    in_=io[nm][256:512, :].rearrange("(c p) n -> p c n", p=128),
                )
            # bulk order matches the output-group-outer consumption order:
            # all of wq first (Q groups run first), then consts (rope for
            # s-block 0 starts right after the Q groups), then wk, wv, wvr
            for c4 in range(1, NPAIR // 2):
                sl = slice(c4 * 512, (c4 + 1) * 512)
                nc.gpsimd.dma_start(
                    out=wq_sb[:, c4 * 4 : c4 * 4 + 4, :],
                    in_=io["wq8"][sl, :].rearrange("(c p) n -> p c n", p=128),
                )
            nc.gpsimd.dma_start(out=ident, in_=io["ident"][:, :])
            nc.gpsimd.dma_start(out=cos2, in_=io["cos2"][:, :])
            nc.gpsimd.dma_start(out=sin2, in_=io["sin2"][:, :])
            nc.gpsimd.dma_start(out=mskT, in_=io["mskT"][:, :])
            nc.gpsimd.dma_start(out=mskR, in_=io["mskR"][:, :, :])
            for w_sb, nm in (
                (wk_sb, "wk8"),
                (wv_sb, "wv8"),
                (wvr_sb, "wvr8"),
            ):
                for c4 in range(1, NPAIR // 2):
                    sl = slice(c4 * 512, (c4 + 1) * 512)
                    nc.gpsimd.dma_start(
                        out=w_sb[:, c4 * 4 : c4 * 4 + 4, :],
                        in_=io[nm][sl, :].rearrange("(c p) n -> p c n", p=128),
                    )

            for sb in range(NB):
                ssl = slice(sb * SB, (sb + 1) * SB)
                ps_q = [
                    psA.tile(
                        [128, SB],
                        F32,
                        name=f"psq{t}_{sb}",
                        tag=f"psq{t}",
                        bufs=1,
                    )
                    for t in range(HQL)
                ]
                ps_k = psA.tile([128, SB], F32, tag="psk")
                ps_v = psA.tile([128, SB], F32, tag="psv")
                # load all 16 pairs up front, then run each output's psum
                # accumulation group back-to-back (single-group marginal cost
                # on the PE is ~20% cheaper than 6-way group interleave)
                xts = []
                for c in range(NPAIR):
                    xt = xpool.tile([128, 4, SB], FP8, name=f"xt{sb}_{c}", tag="xt")
                    rsl = slice(c * 256, (c + 1) * 256)
                    nc.sync.dma_start(
                        out=xt[:, 0:2, :],
                        in_=io["x8"][rsl, ssl].rearrange("(c p) n -> p c n", p=128),
                    )
                    nc.scalar.dma_start(
                        out=xt[:, 2:4, :],
                        in_=io["xr8"][rsl, ssl].rearrange("(c p) n -> p c n", p=128),
                    )
                    xts.append(xt)
                for t in range(HQL):
                    for c in range(NPAIR):
                        nc.tensor.matmul(
                            ps_q[t],
                            lhsT=wq_sb[:, 2 * c : 2 * c + 2, t * 128 : (t + 1) * 128],
                            rhs=xts[c][:, 0:2, :],
                            start=c == 0,
                            stop=c == NPAIR - 1,
                            perf_mode=DR,
                        )
                for c in range(NPAIR):
                    nc.tensor.matmul(
                        ps_k, lhsT=wk_sb[:, 2 * c : 2 * c + 2, :],
                        rhs=xts[c][:, 0:2, :],
                        start=c == 0, stop=c == NPAIR - 1, perf_mode=DR,
                    )
                for c in range(NPAIR):
                    wsl = slice(2 * c, 2 * c + 2)
                    nc.tensor.matmul(
                        ps_v, lhsT=wv_sb[:, wsl, :], rhs=xts[c][:, 0:2, :],
                        start=c == 0, stop=False, perf_mode=DR,
                    )
                    nc.tensor.matmul(
                        ps_v, lhsT=wvr_sb[:, wsl, :], rhs=xts[c][:, 0:2, :],
                        start=False, stop=False, perf_mode=DR,
                    )
                    nc.tensor.matmul(
                        ps_v, lhsT=wv_sb[:, wsl, :], rhs=xts[c][:, 2:4, :],
                        start=False, stop=c == NPAIR - 1, perf_mode=DR,
                    )

                # V^T -> V (PE transpose per 128-col chunk)
                vts = rpool.tile([128, SB], BF16, name=f"vts{sb}", tag="vts")
                nc.scalar.copy(vts, ps_v)
                for u in range(SB // 128):
                    ps_vt = psA.tile(
                        [128, 128], BF16, name=f"psvt{sb}_{u}", tag="psvt", bufs=2
                    )
                    nc.tensor.transpose(
                        ps_vt, vts[:, u * 128 : (u + 1) * 128], ident
                    )
                    nc.vector.tensor_copy(vs_sb[sb][:, u, :], ps_vt)

                # RoPE -> fp8: rot(q) = q*cos2 + pairswap(q)*sin2, all bf16
                # muls, fp8 destination. cos2/sin2 carry 2^-8.
                def rope(ps, dst, idx):
                    qc = rpool.tile([128, SB], BF16, name=f"qc{idx}", tag="qc")
                    nc.scalar.copy(qc, ps)
                    sw = rpool.tile([128, SB], BF16, name=f"sw{idx}", tag="sw")
                    nc.vector.stream_shuffle(sw, qc, SWAP_MASK)
                    t1 = rpool.tile([128, SB], BF16, name=f"t1{idx}", tag="t1")
                    nc.vector.tensor_mul(t1, qc, cos2[:, ssl])
                    t2 = rpool.tile([128, SB], BF16, name=f"t2{idx}", tag="t2")
                    nc.vector.tensor_mul(t2, sw, sin2[:, ssl])
                    nc.vector.tensor_add(dst, t1, t2)

                for t in range(HQL):
                    rope(ps_q[t], qt8f[sb][:, t, :], f"q{sb}_{t}")
                rope(ps_k, kt8f[sb], f"k{sb}")

                # split [128, .] -> [64, 2, .] for DoubleRow score matmuls
                for t in range(HQL):
                    nc.gpsimd.dma_start(out=qt8[sb][:, 0, t, :], in_=qt8f[sb][0:64, t, :])
                    nc.gpsimd.dma_start(out=qt8[sb][:, 1, t, :], in_=qt8f[sb][64:128, t, :])
                nc.gpsimd.dma_start(out=kt8[sb][:, 0, :], in_=kt8f[sb][0:64, :])
                nc.gpsimd.dma_start(out=kt8[sb][:, 1, :], in_=kt8f[sb][64:128, :])
                if DEBUG_DUMPS:
                    nc.gpsimd.dma_start(out=io["dbg_qt"][:, sb, :, :], in_=qt8f[sb])
                    nc.gpsimd.dma_start(out=io["dbg_kt"][:, sb, :], in_=kt8f[sb])
                    nc.gpsimd.dma_start(out=io["dbg_vs"][:, sb, :, :], in_=vs_sb[sb])

        # wo loads fill DMA idle time during stage B
        wo_pool = ctx.enter_context(tc.tile_pool(name="wo_pool", bufs=1))
        wo_sb = wo_pool.tile([128, 2 * NPAIR, HQL * HD], BF16)
        for c4 in range(NPAIR // 2):
            sl = slice(c4 * 512, (c4 + 1) * 512)
            nc.gpsimd.dma_start(
                out=wo_sb[:, c4 * 4 : c4 * 4 + 4, :],
                in_=io["wo"][sl, :].rearrange("(c p) n -> p c n", p=128),
            )

        apool = ctx.enter_context(tc.tile_pool(name="apool", bufs=6))
        opool = ctx.enter_context(tc.tile_pool(name="opool", bufs=4))

        # ================= Stage B: attention (qb outer) =================
        # Unit = 2 sk-chunks sharing one 2-bank PSUM tile -> one wide exp.
        # Pipeline: PV of unit u-2 is emitted after exp of unit u so the PE
        # never waits on ACT latency. Causal mask applied on the PE (extra
        # mskT x mskR[td] matmul into the scores PSUM on diagonal chunks).
        # Denominator: DVE and GpSimd each own a bf16 accumulator (even/odd
        # chunk of each unit); two ones(2^14)-matmuls finish it.
        with ctx_pools_b(tc) as (ppool, spool, psB):
            for qb in range(NB):
                for h in range(HQL):
                    nkc = (qb + 1) * (SB // 128)
                    nu = nkc // 2
                    ps_o = psB.tile(
                        [128, SB], F32, name=f"pso{h}_{qb}", tag="pso", bufs=2
                    )
                    ps_n = psB.tile(
                        [128, SB], F32, name=f"psn{h}_{qb}", tag="psn", bufs=2
                    )
                    pts = {}

                    def consume_u(u, last, h=h, qb=qb, ps_o=ps_o, ps_n=ps_n,
                                  pts=pts):
                        pt2 = pts.pop(u)
                        for v in range(2):
                            kc = 2 * u + v
                            nc.tensor.matmul(
                                ps_o,
                                lhsT=vs_sb[kc // 4][:, kc % 4, :],
                                rhs=pt2[:, v, :],
                                start=kc == 0,
                                stop=last and v == 1,
                            )
                            # denominator: ones(2^14)-matmul accumulates the
                            # partition sum of exp, pre-broadcast + descaled
                            nc.tensor.matmul(
                                ps_n,
                                lhsT=ones_mat,
                                rhs=pt2[:, v, :],
                                start=kc == 0,
                                stop=last and v == 1,
                            )

                    for u in range(nu):
                        ps_s = psB.tile(
                            [128, 2, SB], F32, name=f"pss{h}_{qb}_{u}", tag="pss",
                            bufs=2,
                        )
                        for v in range(2):
                            kc = 2 * u + v
                            td = kc - qb * 4
                            nc.tensor.matmul(
                                ps_s[:, v, :],
                                lhsT=kt8[kc // 4][
                                    :, :, (kc % 4) * 128 : (kc % 4 + 1) * 128
                                ],
                                rhs=qt8[qb][:, :, h, :],
                                start=True,
                                stop=td < 0,
                                perf_mode=DR,
                            )
                            if td >= 0:
                                nc.tensor.matmul(
                                    ps_s[:, v, :],
                                    lhsT=mskT,
                                    rhs=mskR[:, td, :],
                                    start=False,
                                    stop=True,
                                )
                        pt2 = ppool.tile(
                            [128, 2, SB], BF16, name=f"pt{h}_{qb}_{u}", tag="pt"
                        )
                        nc.scalar.activation(
                            pt2, ps_s, mybir.ActivationFunctionType.Exp,
                            scale=EXP_SCALE,
                        )
                        pts[u] = pt2
                        if u >= 2:
                            consume_u(u - 2, last=False)
                    for u in range(max(0, nu - 2), nu):
                        consume_u(u, last=u == nu - 1)

                    rb = spool.tile([128, SB], F32, name=f"rb{h}_{qb}", tag="rb")
                    nc.vector.reciprocal_approx_fast(rb, ps_n)
                    ao = spool.tile(
                        [128, SB], BF16, name=f"ao{h}_{qb}", tag="ao", bufs=4
                    )
                    nc.vector.tensor_mul(ao, ps_o, rb)
                    # sync queue: the gpsimd queue is busy with collectives,
                    # which would backpressure ao -> DVE -> PSUM -> PE
                    nc.sync.dma_start(
                        out=attn_loc[qb][h * 128 : (h + 1) * 128, :], in_=ao
                    )
                if DEBUG_DUMPS and qb == 0:
                    nc.gpsimd.dma_start(
                        out=io["dbg_loc"][:, :], in_=attn_loc[0][:, :]
                    )
                nc.gpsimd.collective_compute(
                    "AllGather",
                    mybir.AluOpType.bypass,
                    replica_groups=[list(range(NCORES))],
                    ins=[attn_loc[qb].opt()],
                    outs=[attn_g[qb].opt()],
                )

        # ============ Stage D: out = attn @ wo (bf16, column shard) ==========
        # Gathered row 128*c of half g2 = contraction chunk c (= q-head 4i+j
        # of core i); wo chunk c rows match. Plain bf16 matmuls, 4 outputs.
        # tile_wait_until: keep the scheduler from hoisting these gather-
        # dependent loads into the stage-B queue regions (an unsatisfied DMA
        # at a queue head blocks every instruction behind it)
        with tc.tile_pool(name="psD", bufs=2, space="PSUM") as psD, \
                tc.tile_wait_until(0.3):
            for g in range(NB):
                osl = slice(g * SB, (g + 1) * SB)
                ats = []
                for q in range(4):
                    at = apool.tile([128, 8, SB], BF16, name=f"at{g}_{q}", tag="at")
                    # not gpsimd: a collective trigger occupies that queue for
                    # the whole CC duration and would delay these loads
                    eng = nc.sync if q % 2 == 0 else nc.scalar
                    eng.dma_start(
                        out=at,
                        in_=attn_g[g][q * 1024 : (q + 1) * 1024, :].rearrange(
                            "(c p) n -> p c n", p=128
                        ),
                    )
                    ats.append(at)
                for n in range(HQL):
                    nsl = slice(n * 128, (n + 1) * 128)
                    ps_d = psD.tile([128, SB], F32, name=f"psd{g}_{n}", tag="psd")
                    for c in range(2 * NPAIR):
                        nc.tensor.matmul(
                            ps_d, lhsT=wo_sb[:, c, nsl], rhs=ats[c // 8][:, c % 8, :],
                            start=c == 0, stop=c == 2 * NPAIR - 1,
                        )
                    ot = opool.tile([128, SB], F32, name=f"ot{g}_{n}", tag="ot")
                    nc.scalar.copy(ot, ps_d)
                    nc.scalar.dma_start(
                        out=io["outT"][n * 128 : (n + 1) * 128, osl], in_=ot
                    )


from contextlib import contextmanager


@contextmanager
def ctx_pools(tc):
    with (
        tc.tile_pool(name="wpool", bufs=1) as wpool,
        tc.tile_pool(name="xpool", bufs=18) as xpool,
        tc.tile_pool(name="rpool", bufs=3) as rpool,
        tc.tile_pool(name="psA", bufs=1, space="PSUM") as psA,
    ):
        yield wpool, xpool, rpool, psA


@contextmanager
def ctx_pools_b(tc):
    with (
        tc.tile_pool(name="ppool", bufs=8) as ppool,
        tc.tile_pool(name="spool", bufs=2) as spool,
        tc.tile_pool(name="psB", bufs=2, space="PSUM") as psB,
    ):
        yield ppool, spool, psB


_NC_CACHE = None


def _get_nc():
    global _NC_CACHE
    if _NC_CACHE is None:
        _NC_CACHE = _build_nc()
    return _NC_CACHE


def _prep_in_maps(x, freqs_cos, freqs_sin, wq, wk, wv, wo):
    bf = ml_dtypes.bfloat16
    f8 = ml_dtypes.float8_e4m3
    S7 = 128.0

    x = np.asarray(x, np.float32).reshape(S, D)
    xT = np.ascontiguousarray(x.T) * S7
    x8 = xT.astype(f8)
    xr8 = (xT - x8.astype(np.float32)).astype(f8)

    cos = np.asarray(freqs_cos, np.float32)  # [S, HD/2]
    sin = np.asarray(freqs_sin, np.float32)
    cos2 = np.repeat(cos.T, 2, axis=0)  # [HD, S]
    sin_t = sin.T
    sin2 = np.empty((HD, S), np.float32)
    sin2[0::2] = -sin_t
    sin2[1::2] = sin_t
    rs = 1.0 / 256.0  # 2^-8: descale 2^-14, rescale 2^6 for fp8 q/k
    cos2 = cos2 * rs
    sin2 = sin2 * rs

    # causal mask as matmul: mskT.T @ mskR[td] = -1e6 where c < 128*td + p
    kk = np.arange(128)
    mskT = (kk[:, None] <= kk[None, :]).astype(bf)  # [k, p] lower-inclusive
    mskR = np.zeros((128, NB, SB), np.float32)
    for td in range(NB):
        cc = np.arange(SB)
        hit = (cc[None, :] == 128 * td + kk[:, None] - 1).astype(np.float32)
        hit[0, :] += (cc < 128 * td).astype(np.float32)
        mskR[:, td, :] = -1e6 * hit
    ident = np.eye(128, dtype=bf)

    wq = np.asarray(wq, np.float32) * S7
    wk = np.asarray(wk, np.float32) * S7
    wv = np.asarray(wv, np.float32) * S7
    wo = np.asarray(wo, np.float32)
    in_maps = []
    for i in range(NCORES):
        wq_i = np.ascontiguousarray(wq[:, i * HQL * HD : (i + 1) * HQL * HD])
        wk_i = np.ascontiguousarray(wk[:, i * HD : (i + 1) * HD])
        wv_i = np.ascontiguousarray(wv[:, i * HD : (i + 1) * HD])
        wo_i = np.ascontiguousarray(wo[:, i * HQL * HD : (i + 1) * HQL * HD])
        wv8 = wv_i.astype(f8)
        wvr8 = (wv_i - wv8.astype(np.float32)).astype(f8)
        in_maps.append(
            {
                "x8": x8,
                "xr8": xr8,
                "cos2": cos2.astype(bf),
                "sin2": sin2.astype(bf),
                "mskT": mskT,
                "mskR": mskR.astype(bf),
                "ident": ident,
                "wq8": wq_i.astype(f8),
                "wk8": wk_i.astype(f8),
                "wv8": wv8,
                "wvr8": wvr8,
                "wo": wo_i.astype(bf),
            }
        )
    return in_maps


def _install_trace_shims():
    """The container's antenv lacks axon_hooks; replicate trn_boot's ctypes
    NTFF hook so run_bass_kernel_spmd(trace=True) works. Also stub out the
    fish-bucket artifact upload (no bucket access here)."""
    import sys
    import types
    import ctypes
    import contextlib

    if "antenv.axon_hooks" not in sys.modules:
        mod = types.ModuleType("antenv.axon_hooks")
        mod._hook = None

        def set_axon_ntff_profile_hook(h):
            mod._hook = h

        def get_axon_ntff_profile_hook():
            return mod._hook

        mod.set_axon_ntff_profile_hook = set_axon_ntff_profile_hook
        mod.get_axon_ntff_profile_hook = get_axon_ntff_profile_hook
        sys.modules["antenv.axon_hooks"] = mod

        so_path = "/opt/axon/libaxon_pjrt.so"
        lib = ctypes.CDLL(so_path)
        if hasattr(lib, "axon_start_nrt_profile"):
            lib.axon_start_nrt_profile.argtypes = [
                ctypes.POINTER(ctypes.c_int64),
                ctypes.c_size_t,
            ]
            lib.axon_start_nrt_profile.restype = ctypes.c_int64
            lib.axon_stop_nrt_profile.argtypes = [ctypes.c_char_p]
            lib.axon_stop_nrt_profile.restype = ctypes.c_int64

            @contextlib.contextmanager
            def _hook(output_dir, device_ids):
                import jax

                jax.devices()
                if device_ids:
                    ids = (ctypes.c_int64 * len(device_ids))(*device_ids)
                    rc = lib.axon_start_nrt_profile(ids, len(device_ids))
                else:
                    rc = lib.axon_start_nrt_profile(None, 0)
                if rc != 0:
                    raise RuntimeError(f"axon_start_nrt_profile rc={rc}")
                try:
                    yield
                finally:
                    n = lib.axon_stop_nrt_profile(str(output_dir).encode())
                    if n <= 0:
                        print(f"WARNING: axon_stop_nrt_profile rc={n}")

            set_axon_ntff_profile_hook(_hook)

    import concourse.bass_utils as bu

    bu.upload_artifacts = lambda tmpdir: "local://" + str(tmpdir)


def run(inputs, trace=False, **kw):
    nc = _get_nc()
    if trace:
        _install_trace_shims()
    in_maps = _prep_in_maps(**inputs)
    res = run_bass_kernel_spmd(nc, in_maps, list(range(NCORES)), trace=trace, **kw)
    out = np.concatenate(
        [res.results[i]["outT"].T for i in range(NCORES)], axis=1
    )
    return out.reshape(B, S, D).astype(np.float32), res


def kernel(x, freqs_cos, freqs_sin, wq, wk, wv, wo):
    out, _ = run(
        dict(
            x=x,
            freqs_cos=freqs_cos,
            freqs_sin=freqs_sin,
            wq=wq,
            wk=wk,
            wv=wv,
            wo=wo,
        )
    )
    return out


# revision 52
# speedup vs baseline: 1.0175x; 1.0175x over previous
"""GQA attention + RoPE + causal softmax + output projection on 8 TRN2 cores.

Sharding: tensor-parallel over heads. Core i owns q-heads [4i, 4i+4) and
kv-head i (GQA group size 4 == HQ/8, HK/8 = 1).

fp8 strategy (keeps rel err ~1%, well under the 2e-2 gate):
  - Q and K paths run PURE fp8e4 (x, wq, wk, and the rope'd Q^T/K^T all fp8):
    score errors are ~5% of |s| with |s| ~ 4e-3, so p = exp(s) moves by
    ~2e-4 absolute -- diluted to ~0.03% on the output by the softmax.
  - V path and the output projection carry first-order residual corrections:
    x = x8 + xr8, wv = wv8 + wvr8, attn = ao8 + aor8, wo = wo8 + wor8 (all
    fp8 pairs; the resid*resid cross term is dropped, ~0.4% second order).
  - All fp8 matmuls use MatmulPerfMode.DoubleRow (2 contraction subtiles per
    instruction at 0.5 cycles/output-column = 4x bf16 throughput). The
    HD=128 score contraction is split as [64 partitions x 2 subtiles].
  - P*V stays bf16 (p values cluster at 1.0; fp8 would quantize away the
    softmax signal).

Scale bookkeeping (powers of two, folded into existing constants):
  x8,w8 carry 2^7 each -> projection PSUM = 2^14 * true.
  cos2/sin2 carry 2^-8   -> Q^T/K^T fp8 = 2^6 * true; score PSUM = 2^12 * s.
  exp scale = 2^-12/sqrt(HD).  V stays scaled: vts = 2^14 * v.
  ones_mat = 16 = 2^(14-10)  -> ao = 2^10 * attn (good fp8 range).
  out PSUM = 2^(10+7) * true -> final ACT copy uses scale 2^-17.

Softmax denominator: DVE accumulates exp chunks into two bf16 accumulators
(even/odd chunks, so the serial add chain keeps up with the PE), then two
ones-matmuls fold the partition sum + broadcast + 2^4 scale in one step.

Collectives: two AllGathers over sequence halves (all 4 heads + resid rows
per half), issued after qb=1 and qb=3 of the qb-outer attention loop; the
output projection consumes half 0 while half 1 is still being gathered.
"""

import os

import numpy as np
import ml_dtypes

import concourse.bass as bass
import concourse.mybir as mybir
import concourse.tile as tile
from concourse import bacc
from concourse.bass_utils import run_bass_kernel_spmd

# Problem dims (hardcoded per contract)
B, S, D = 1, 2048, 4096
HQ, HK, HD = 32, 8, 128
NCORES = 8
HQL = HQ // NCORES          # 4 local q heads
SB = 512                    # seq block (matmul moving free dim)
NB = S // SB                # 4 seq blocks
NPAIR = D // 256            # 16 DoubleRow contraction pairs for D
SCALE = 1.0 / float(np.sqrt(HD))
EXP_SCALE = SCALE / 4096.0  # scores PSUM carries 2^12
S2 = S // 2                 # gather half width

F32 = mybir.dt.float32
BF16 = mybir.dt.bfloat16
FP8 = mybir.dt.float8e4
DR = mybir.MatmulPerfMode.DoubleRow
DEBUG_DUMPS = os.environ.get("BASSDBG", "") == "1"

# stream_shuffle mask: swap adjacent pairs within each 32-partition quadrant
SWAP_MASK = [(i ^ 1) for i in range(32)]


def _build_nc():
    nc = bacc.Bacc(
        "TRN2", target_bir_lowering=False, debug=False, num_devices=NCORES
    )

    io = {}
    io["x8"] = nc.dram_tensor("x8", [D, S], FP8, kind="ExternalInput")
    io["xbf"] = nc.dram_tensor("xbf", [D, S], BF16, kind="ExternalInput")
    io["wq8"] = nc.dram_tensor("wq8", [D, HQL * HD], FP8, kind="ExternalInput")
    io["wk8"] = nc.dram_tensor("wk8", [D, HD], FP8, kind="ExternalInput")
    io["wv"] = nc.dram_tensor("wv", [D, HD], BF16, kind="ExternalInput")
    io["wo"] = nc.dram_tensor("wo", [D, HQL * HD], BF16, kind="ExternalInput")
    io["cos2"] = nc.dram_tensor("cos2", [HD, S], BF16, kind="ExternalInput")
    io["sin2"] = nc.dram_tensor("sin2", [HD, S], BF16, kind="ExternalInput")
    # causal mask as a matmul: T8 lower-inclusive triangle (stationary) and
    # per-td moving panels R with -1e6 markers; T8.T @ R[td] adds -1e6 to
    # every (p, c) with c < 128*td + p, exactly the invalid region.
    io["mskT"] = nc.dram_tensor("mskT", [128, 128], BF16, kind="ExternalInput")
    io["mskR"] = nc.dram_tensor("mskR", [128, NB, SB], BF16, kind="ExternalInput")
    io["ident"] = nc.dram_tensor("ident", [128, 128], BF16, kind="ExternalInput")
    io["outT"] = nc.dram_tensor("outT", [HQL * HD, S], F32, kind="ExternalOutput")
    if DEBUG_DUMPS:
        io["dbg_qt"] = nc.dram_tensor(
            "dbg_qt", [128, NB, HQL, SB], FP8, kind="ExternalOutput"
        )
        io["dbg_kt"] = nc.dram_tensor(
            "dbg_kt", [128, NB, SB], FP8, kind="ExternalOutput"
        )
        io["dbg_vs"] = nc.dram_tensor(
            "dbg_vs", [128, NB, SB // 128, HD], BF16, kind="ExternalOutput"
        )
        io["dbg_loc"] = nc.dram_tensor(
            "dbg_loc", [HQL * HD, SB], BF16, kind="ExternalOutput"
        )

    with tile.TileContext(nc) as tc:
        _body(tc, io)
    nc.compile()
    return nc


def _body(tc, io):
    nc = tc.nc
    from contextlib import ExitStack

    ctx = ExitStack()
    with ctx:
        consts = ctx.enter_context(tc.tile_pool(name="consts", bufs=1))
        qkv = ctx.enter_context(tc.tile_pool(name="qkv", bufs=1))
        dram = ctx.enter_context(tc.tile_pool(name="dram", bufs=1, space="DRAM"))

        cos2 = consts.tile([HD, S], BF16)
        sin2 = consts.tile([HD, S], BF16)
        ident = consts.tile([128, 128], BF16)
        mskT = consts.tile([128, 128], BF16)
        mskR = consts.tile([128, NB, SB], BF16)
        # ones * 2^14: the denominator matmul folds partition-sum, broadcast
        # and the 2^14 V-path descale in one shot -> ao lands at true scale
        ones_mat = consts.tile([128, 128], BF16)
        nc.vector.memset(ones_mat, 16384.0)

        # persistent per-core tensors, split per s-block for fine-grained deps
        qt8f = [
            qkv.tile([128, HQL, SB], FP8, name=f"qt8f{sb}") for sb in range(NB)
        ]
        kt8f = [qkv.tile([128, SB], FP8, name=f"kt8f{sb}") for sb in range(NB)]
        # [64, 2, ...] split layouts for DoubleRow score matmuls
        qt8 = [
            qkv.tile([64, 2, HQL, SB], FP8, name=f"qt8_{sb}") for sb in range(NB)
        ]
        kt8 = [qkv.tile([64, 2, SB], FP8, name=f"kt8_{sb}") for sb in range(NB)]
        vs_sb = [
            qkv.tile([128, SB // 128, HD], BF16, name=f"vs{sb}") for sb in range(NB)
        ]

        # quarter-sequence bounce + gather buffers (bf16, 4 heads per qb
        # block); gathered row 128*c = contraction chunk c = q-head 4i+j
        attn_loc = [
            dram.tile([HQL * HD, SB], BF16, name=f"attn_loc{g2}")
            for g2 in range(NB)
        ]
        attn_g = [
            dram.tile(
                [NCORES * HQL * HD, SB],
                BF16,
                name=f"attn_g{g2}",
                addr_space="Shared",
            )
            for g2 in range(NB)
        ]

        # ================= Stage A: projections + RoPE =================
        with ctx_pools(tc) as (wpool, xpool, rpool, psA):
            # warm-up DoubleRow (discarded): the first dual-fp8 ldweights in a
            # program mis-executes (partial-NaN psum); absorb it on zeros.
            dmy_l = wpool.tile([128, 2, 128], FP8, name="dmy_l")
            dmy_r = wpool.tile([128, 2, 8], FP8, name="dmy_r")
            nc.vector.memset(dmy_l, 0.0)
            nc.vector.memset(dmy_r, 0.0)
            ps_warm = psA.tile([128, 8], F32, name="ps_warm", tag="psvt", bufs=2)
            nc.tensor.matmul(
                ps_warm, lhsT=dmy_l, rhs=dmy_r, start=True, stop=True, perf_mode=DR
            )

            wq_sb = wpool.tile([128, 2 * NPAIR, HQL * HD], FP8)
            wk_sb = wpool.tile([128, 2 * NPAIR, HD], FP8)
            wv_sb = wpool.tile([128, 2 * NPAIR, HD], BF16)
            # first pair as fine slices so the PE can start ASAP
            for t in range(HQL):
                nc.gpsimd.dma_start(
                    out=wq_sb[:, 0:2, t * 128 : (t + 1) * 128],
                    in_=io["wq8"][0:256, t * 128 : (t + 1) * 128].rearrange(
                        "(c p) n -> p c n", p=128
                    ),
                )
            nc.gpsimd.dma_start(
                out=wk_sb[:, 0:2, :],
                in_=io["wk8"][0:256, :].rearrange("(c p) n -> p c n", p=128),
            )
            nc.gpsimd.dma_start(
                out=wv_sb[:, 0:2, :],
                in_=io["wv"][0:256, :].rearrange("(c p) n -> p c n", p=128),
            )
            # pair 1 (the c4 bulk loop below starts at chunk 4)
            for w_sb, nm in (
                (wq_sb, "wq8"),
                (wk_sb, "wk8"),
                (wv_sb, "wv"),
            ):
                nc.gpsimd.dma_start(
                    out=w_sb[:, 2:4, :],
                    in_=io[nm][256:512, :].rearrange("(c p) n -> p c n", p=128),
                )
            # bulk order matches the output-group-outer consumption order:
            # all of wq first (Q groups run first), then consts (rope for
            # s-block 0 starts right after the Q groups), then wk, wv
            for c4 in range(1, NPAIR // 2):
                sl = slice(c4 * 512, (c4 + 1) * 512)
                nc.gpsimd.dma_start(
                    out=wq_sb[:, c4 * 4 : c4 * 4 + 4, :],
                    in_=io["wq8"][sl, :].rearrange("(c p) n -> p c n", p=128),
                )
            nc.gpsimd.dma_start(out=ident, in_=io["ident"][:, :])
            nc.gpsimd.dma_start(out=cos2, in_=io["cos2"][:, :])
            nc.gpsimd.dma_start(out=sin2, in_=io["sin2"][:, :])
            nc.gpsimd.dma_start(out=mskT, in_=io["mskT"][:, :])
            nc.gpsimd.dma_start(out=mskR, in_=io["mskR"][:, :, :])
            for w_sb, nm in ((wk_sb, "wk8"), (wv_sb, "wv")):
                for c4 in range(1, NPAIR // 2):
                    sl = slice(c4 * 512, (c4 + 1) * 512)
                    nc.gpsimd.dma_start(
                        out=w_sb[:, c4 * 4 : c4 * 4 + 4, :],
                        in_=io[nm][sl, :].rearrange("(c p) n -> p c n", p=128),
                    )

            for sb in range(NB):
                ssl = slice(sb * SB, (sb + 1) * SB)
                ps_q = [
                    psA.tile(
                        [128, SB],
                        F32,
                        name=f"psq{t}_{sb}",
                        tag=f"psq{t}",
                        bufs=1,
                    )
                    for t in range(HQL)
                ]
                ps_k = psA.tile([128, SB], F32, tag="psk")
                ps_v = psA.tile([128, SB], F32, tag="psv")
                # load all 16 pairs up front, then run each output's psum
                # accumulation group back-to-back (single-group marginal cost
                # on the PE is ~20% cheaper than 6-way group interleave)
                # x8 pair quads on sync (Q/K fp8 DR); xbf quads on scalar
                # (V in full bf16 -- no residual streams, fewer instructions)
                x8q, xbq = [], []
                for q4 in range(NPAIR // 4):
                    xq = xpool.tile([128, 8, SB], FP8, name=f"x8q{sb}_{q4}", tag="x8q")
                    rsl = slice(q4 * 1024, (q4 + 1) * 1024)
                    nc.sync.dma_start(
                        out=xq,
                        in_=io["x8"][rsl, ssl].rearrange("(c p) n -> p c n", p=128),
                    )
                    x8q.append(xq)
                    xb = xpool.tile(
                        [128, 8, SB], BF16, name=f"xbq{sb}_{q4}", tag="xbq"
                    )
                    nc.scalar.dma_start(
                        out=xb,
                        in_=io["xbf"][rsl, ssl].rearrange("(c p) n -> p c n", p=128),
                    )
                    xbq.append(xb)

                def x8p(c):
                    return x8q[c // 4][:, (c % 4) * 2 : (c % 4) * 2 + 2, :]

                for t in range(HQL):
                    for c in range(NPAIR):
                        nc.tensor.matmul(
                            ps_q[t],
                            lhsT=wq_sb[:, 2 * c : 2 * c + 2, t * 128 : (t + 1) * 128],
                            rhs=x8p(c),
                            start=c == 0,
                            stop=c == NPAIR - 1,
                            perf_mode=DR,
                        )
                for c in range(NPAIR):
                    nc.tensor.matmul(
                        ps_k, lhsT=wk_sb[:, 2 * c : 2 * c + 2, :], rhs=x8p(c),
                        start=c == 0, stop=c == NPAIR - 1, perf_mode=DR,
                    )
                for c in range(2 * NPAIR):
                    nc.tensor.matmul(
                        ps_v, lhsT=wv_sb[:, c, :], rhs=xbq[c // 8][:, c % 8, :],
                        start=c == 0, stop=c == 2 * NPAIR - 1,
                    )

                # V^T -> V (PE transpose per 128-col chunk)
                vts = rpool.tile([128, SB], BF16, name=f"vts{sb}", tag="vts")
                nc.scalar.copy(vts, ps_v)
                for u in range(SB // 128):
                    ps_vt = psA.tile(
                        [128, 128], BF16, name=f"psvt{sb}_{u}", tag="psvt", bufs=2
                    )
                    nc.tensor.transpose(
                        ps_vt, vts[:, u * 128 : (u + 1) * 128], ident
                    )
                    nc.vector.tensor_copy(vs_sb[sb][:, u, :], ps_vt)

                # RoPE -> fp8: rot(q) = q*cos2 + pairswap(q)*sin2, all bf16
                # muls, fp8 destination. cos2/sin2 carry 2^-8.
                def rope(ps, dst, idx):
                    qc = rpool.tile([128, SB], BF16, name=f"qc{idx}", tag="qc")
                    nc.scalar.copy(qc, ps)
                    sw = rpool.tile([128, SB], BF16, name=f"sw{idx}", tag="sw")
                    nc.vector.stream_shuffle(sw, qc, SWAP_MASK)
                    t1 = rpool.tile([128, SB], BF16, name=f"t1{idx}", tag="t1")
                    nc.vector.tensor_mul(t1, qc, cos2[:, ssl])
                    t2 = rpool.tile([128, SB], BF16, name=f"t2{idx}", tag="t2")
                    nc.vector.tensor_mul(t2, sw, sin2[:, ssl])
                    nc.vector.tensor_add(dst, t1, t2)

                for t in range(HQL):
                    rope(ps_q[t], qt8f[sb][:, t, :], f"q{sb}_{t}")
                rope(ps_k, kt8f[sb], f"k{sb}")

                # split [128, .] -> [64, 2, .] for DoubleRow score matmuls
                for t in range(HQL):
                    nc.gpsimd.dma_start(out=qt8[sb][:, 0, t, :], in_=qt8f[sb][0:64, t, :])
                    nc.gpsimd.dma_start(out=qt8[sb][:, 1, t, :], in_=qt8f[sb][64:128, t, :])
                nc.gpsimd.dma_start(out=kt8[sb][:, 0, :], in_=kt8f[sb][0:64, :])
                nc.gpsimd.dma_start(out=kt8[sb][:, 1, :], in_=kt8f[sb][64:128, :])
                if DEBUG_DUMPS:
                    nc.gpsimd.dma_start(out=io["dbg_qt"][:, sb, :, :], in_=qt8f[sb])
                    nc.gpsimd.dma_start(out=io["dbg_kt"][:, sb, :], in_=kt8f[sb])
                    nc.gpsimd.dma_start(out=io["dbg_vs"][:, sb, :, :], in_=vs_sb[sb])

        # wo loads fill DMA idle time during stage B
        wo_pool = ctx.enter_context(tc.tile_pool(name="wo_pool", bufs=1))
        wo_sb = wo_pool.tile([128, 2 * NPAIR, HQL * HD], BF16)
        for c4 in range(NPAIR // 2):
            sl = slice(c4 * 512, (c4 + 1) * 512)
            nc.gpsimd.dma_start(
                out=wo_sb[:, c4 * 4 : c4 * 4 + 4, :],
                in_=io["wo"][sl, :].rearrange("(c p) n -> p c n", p=128),
            )

        apool = ctx.enter_context(tc.tile_pool(name="apool", bufs=6))
        opool = ctx.enter_context(tc.tile_pool(name="opool", bufs=4))

        # ================= Stage B: attention (qb outer) =================
        # Unit = 2 sk-chunks sharing one 2-bank PSUM tile -> one wide exp.
        # Pipeline: PV of unit u-2 is emitted after exp of unit u so the PE
        # never waits on ACT latency. Causal mask applied on the PE (extra
        # mskT x mskR[td] matmul into the scores PSUM on diagonal chunks).
        # Denominator: DVE and GpSimd each own a bf16 accumulator (even/odd
        # chunk of each unit); two ones(2^14)-matmuls finish it.
        with ctx_pools_b(tc) as (ppool, spool, psB):
            for qb in range(NB):
                for h in range(HQL):
                    nkc = (qb + 1) * (SB // 128)
                    nu = nkc // 2
                    ps_o = psB.tile(
                        [128, SB], F32, name=f"pso{h}_{qb}", tag="pso", bufs=2
                    )
                    ps_n = psB.tile(
                        [128, SB], F32, name=f"psn{h}_{qb}", tag="psn", bufs=2
                    )
                    pts = {}

                    def consume_u(u, last, h=h, qb=qb, ps_o=ps_o, ps_n=ps_n,
                                  pts=pts):
                        pt2 = pts.pop(u)
                        for v in range(2):
                            kc = 2 * u + v
                            nc.tensor.matmul(
                                ps_o,
                                lhsT=vs_sb[kc // 4][:, kc % 4, :],
                                rhs=pt2[:, v, :],
                                start=kc == 0,
                                stop=last and v == 1,
                            )
                            # denominator: ones(2^14)-matmul accumulates the
                            # partition sum of exp, pre-broadcast + descaled
                            nc.tensor.matmul(
                                ps_n,
                                lhsT=ones_mat,
                                rhs=pt2[:, v, :],
                                start=kc == 0,
                                stop=last and v == 1,
                            )

                    for u in range(nu):
                        ps_s = psB.tile(
                            [128, 2, SB], F32, name=f"pss{h}_{qb}_{u}", tag="pss",
                            bufs=2,
                        )
                        for v in range(2):
                            kc = 2 * u + v
                            td = kc - qb * 4
                            nc.tensor.matmul(
                                ps_s[:, v, :],
                                lhsT=kt8[kc // 4][
                                    :, :, (kc % 4) * 128 : (kc % 4 + 1) * 128
                                ],
                                rhs=qt8[qb][:, :, h, :],
                                start=True,
                                stop=td < 0,
                                perf_mode=DR,
                            )
                            if td >= 0:
                                nc.tensor.matmul(
                                    ps_s[:, v, :],
                                    lhsT=mskT,
                                    rhs=mskR[:, td, :],
                                    start=False,
                                    stop=True,
                                )
                        pt2 = ppool.tile(
                            [128, 2, SB], BF16, name=f"pt{h}_{qb}_{u}", tag="pt"
                        )
                        nc.scalar.activation(
                            pt2, ps_s, mybir.ActivationFunctionType.Exp,
                            scale=EXP_SCALE,
                        )
                        pts[u] = pt2
                        if u >= 2:
                            consume_u(u - 2, last=False)
                    for u in range(max(0, nu - 2), nu):
                        consume_u(u, last=u == nu - 1)

                    rb = spool.tile([128, SB], F32, name=f"rb{h}_{qb}", tag="rb")
                    nc.vector.reciprocal_approx_fast(rb, ps_n)
                    ao = spool.tile(
                        [128, SB], BF16, name=f"ao{h}_{qb}", tag="ao", bufs=4
                    )
                    nc.vector.tensor_mul(ao, ps_o, rb)
                    # sync queue: the gpsimd queue is busy with collectives,
                    # which would backpressure ao -> DVE -> PSUM -> PE
                    nc.sync.dma_start(
                        out=attn_loc[qb][h * 128 : (h + 1) * 128, :], in_=ao
                    )
                if DEBUG_DUMPS and qb == 0:
                    nc.gpsimd.dma_start(
                        out=io["dbg_loc"][:, :], in_=attn_loc[0][:, :]
                    )
                nc.gpsimd.collective_compute(
                    "AllGather",
                    mybir.AluOpType.bypass,
                    replica_groups=[list(range(NCORES))],
                    ins=[attn_loc[qb].opt()],
                    outs=[attn_g[qb].opt()],
                )

        # ============ Stage D: out = attn @ wo (bf16, column shard) ==========
        # Gathered row 128*c of half g2 = contraction chunk c (= q-head 4i+j
        # of core i); wo chunk c rows match. Plain bf16 matmuls, 4 outputs.
        # tile_wait_until: keep the scheduler from hoisting these gather-
        # dependent loads into the stage-B queue regions (an unsatisfied DMA
        # at a queue head blocks every instruction behind it)
        with tc.tile_pool(name="psD", bufs=2, space="PSUM") as psD, \
                tc.tile_wait_until(0.3):
            for g in range(NB):
                osl = slice(g * SB, (g + 1) * SB)
                ats = []
                for q in range(4):
                    at = apool.tile([128, 8, SB], BF16, name=f"at{g}_{q}", tag="at")
                    # not gpsimd: a collective trigger occupies that queue for
                    # the whole CC duration and would delay these loads
                    eng = nc.sync if q % 2 == 0 else nc.scalar
                    eng.dma_start(
                        out=at,
                        in_=attn_g[g][q * 1024 : (q + 1) * 1024, :].rearrange(
                            "(c p) n -> p c n", p=128
                        ),
                    )
                    ats.append(at)
                for n in range(HQL):
                    nsl = slice(n * 128, (n + 1) * 128)
                    ps_d = psD.tile([128, SB], F32, name=f"psd{g}_{n}", tag="psd")
                    for c in range(2 * NPAIR):
                        nc.tensor.matmul(
                            ps_d, lhsT=wo_sb[:, c, nsl], rhs=ats[c // 8][:, c % 8, :],
                            start=c == 0, stop=c == 2 * NPAIR - 1,
                        )
                    ot = opool.tile([128, SB], F32, name=f"ot{g}_{n}", tag="ot")
                    nc.scalar.copy(ot, ps_d)
                    nc.scalar.dma_start(
                        out=io["outT"][n * 128 : (n + 1) * 128, osl], in_=ot
                    )


from contextlib import contextmanager


@contextmanager
def ctx_pools(tc):
    with (
        tc.tile_pool(name="wpool", bufs=1) as wpool,
        tc.tile_pool(name="xpool", bufs=6) as xpool,
        tc.tile_pool(name="rpool", bufs=3) as rpool,
        tc.tile_pool(name="psA", bufs=1, space="PSUM") as psA,
    ):
        yield wpool, xpool, rpool, psA


@contextmanager
def ctx_pools_b(tc):
    with (
        tc.tile_pool(name="ppool", bufs=8) as ppool,
        tc.tile_pool(name="spool", bufs=2) as spool,
        tc.tile_pool(name="psB", bufs=2, space="PSUM") as psB,
    ):
        yield ppool, spool, psB


_NC_CACHE = None


def _get_nc():
    global _NC_CACHE
    if _NC_CACHE is None:
        _NC_CACHE = _build_nc()
    return _NC_CACHE


def _prep_in_maps(x, freqs_cos, freqs_sin, wq, wk, wv, wo):
    bf = ml_dtypes.bfloat16
    f8 = ml_dtypes.float8_e4m3
    S7 = 128.0

    x = np.asarray(x, np.float32).reshape(S, D)
    xT = np.ascontiguousarray(x.T) * S7
    x8 = xT.astype(f8)
    xbf = xT.astype(bf)

    cos = np.asarray(freqs_cos, np.float32)  # [S, HD/2]
    sin = np.asarray(freqs_sin, np.float32)
    cos2 = np.repeat(cos.T, 2, axis=0)  # [HD, S]
    sin_t = sin.T
    sin2 = np.empty((HD, S), np.float32)
    sin2[0::2] = -sin_t
    sin2[1::2] = sin_t
    rs = 1.0 / 256.0  # 2^-8: descale 2^-14, rescale 2^6 for fp8 q/k
    cos2 = cos2 * rs
    sin2 = sin2 * rs

    # causal mask as matmul: mskT.T @ mskR[td] = -1e6 where c < 128*td + p
    kk = np.arange(128)
    mskT = (kk[:, None] <= kk[None, :]).astype(bf)  # [k, p] lower-inclusive
    mskR = np.zeros((128, NB, SB), np.float32)
    for td in range(NB):
        cc = np.arange(SB)
        hit = (cc[None, :] == 128 * td + kk[:, None] - 1).astype(np.float32)
        hit[0, :] += (cc < 128 * td).astype(np.float32)
        mskR[:, td, :] = -1e6 * hit
    ident = np.eye(128, dtype=bf)

    wq = np.asarray(wq, np.float32) * S7
    wk = np.asarray(wk, np.float32) * S7
    wv = np.asarray(wv, np.float32) * S7
    wo = np.asarray(wo, np.float32)
    in_maps = []
    for i in range(NCORES):
        wq_i = np.ascontiguousarray(wq[:, i * HQL * HD : (i + 1) * HQL * HD])
        wk_i = np.ascontiguousarray(wk[:, i * HD : (i + 1) * HD])
        wv_i = np.ascontiguousarray(wv[:, i * HD : (i + 1) * HD])
        wo_i = np.ascontiguousarray(wo[:, i * HQL * HD : (i + 1) * HQL * HD])
        in_maps.append(
            {
                "x8": x8,
                "xbf": xbf,
                "cos2": cos2.astype(bf),
                "sin2": sin2.astype(bf),
                "mskT": mskT,
                "mskR": mskR.astype(bf),
                "ident": ident,
                "wq8": wq_i.astype(f8),
                "wk8": wk_i.astype(f8),
                "wv": wv_i.astype(bf),
                "wo": wo_i.astype(bf),
            }
        )
    return in_maps


def _install_trace_shims():
    """The container's antenv lacks axon_hooks; replicate trn_boot's ctypes
    NTFF hook so run_bass_kernel_spmd(trace=True) works. Also stub out the
    fish-bucket artifact upload (no bucket access here)."""
    import sys
    import types
    import ctypes
    import contextlib

    if "antenv.axon_hooks" not in sys.modules:
        mod = types.ModuleType("antenv.axon_hooks")
        mod._hook = None

        def set_axon_ntff_profile_hook(h):
            mod._hook = h

        def get_axon_ntff_profile_hook():
            return mod._hook

        mod.set_axon_ntff_profile_hook = set_axon_ntff_profile_hook
        mod.get_axon_ntff_profile_hook = get_axon_ntff_profile_hook
        sys.modules["antenv.axon_hooks"] = mod

        so_path = "/opt/axon/libaxon_pjrt.so"
        lib = ctypes.CDLL(so_path)
        if hasattr(lib, "axon_start_nrt_profile"):
            lib.axon_start_nrt_profile.argtypes = [
                ctypes.POINTER(ctypes.c_int64),
                ctypes.c_size_t,
            ]
            lib.axon_start_nrt_profile.restype = ctypes.c_int64
            lib.axon_stop_nrt_profile.argtypes = [ctypes.c_char_p]
            lib.axon_stop_nrt_profile.restype = ctypes.c_int64

            @contextlib.contextmanager
            def _hook(output_dir, device_ids):
                import jax

                jax.devices()
                if device_ids:
                    ids = (ctypes.c_int64 * len(device_ids))(*device_ids)
                    rc = lib.axon_start_nrt_profile(ids, len(device_ids))
                else:
                    rc = lib.axon_start_nrt_profile(None, 0)
                if rc != 0:
                    raise RuntimeError(f"axon_start_nrt_profile rc={rc}")
                try:
                    yield
                finally:
                    n = lib.axon_stop_nrt_profile(str(output_dir).encode())
                    if n <= 0:
                        print(f"WARNING: axon_stop_nrt_profile rc={n}")

            set_axon_ntff_profile_hook(_hook)

    import concourse.bass_utils as bu

    bu.upload_artifacts = lambda tmpdir: "local://" + str(tmpdir)


def run(inputs, trace=False, **kw):
    nc = _get_nc()
    if trace:
        _install_trace_shims()
    in_maps = _prep_in_maps(**inputs)
    res = run_bass_kernel_spmd(nc, in_maps, list(range(NCORES)), trace=trace, **kw)
    out = np.concatenate(
        [res.results[i]["outT"].T for i in range(NCORES)], axis=1
    )
    return out.reshape(B, S, D).astype(np.float32), res


def kernel(x, freqs_cos, freqs_sin, wq, wk, wv, wo):
    out, _ = run(
        dict(
            x=x,
            freqs_cos=freqs_cos,
            freqs_sin=freqs_sin,
            wq=wq,
            wk=wk,
            wv=wv,
            wo=wo,
        )
    )
    return out


# revision 53
# speedup vs baseline: 1.0477x; 1.0297x over previous
"""GQA attention + RoPE + causal softmax + output projection on 8 TRN2 cores.

Sharding: tensor-parallel over heads. Core i owns q-heads [4i, 4i+4) and
kv-head i (GQA group size 4 == HQ/8, HK/8 = 1).

fp8 strategy (keeps rel err ~1%, well under the 2e-2 gate):
  - Q and K paths run PURE fp8e4 (x, wq, wk, and the rope'd Q^T/K^T all fp8):
    score errors are ~5% of |s| with |s| ~ 4e-3, so p = exp(s) moves by
    ~2e-4 absolute -- diluted to ~0.03% on the output by the softmax.
  - V path and the output projection carry first-order residual corrections:
    x = x8 + xr8, wv = wv8 + wvr8, attn = ao8 + aor8, wo = wo8 + wor8 (all
    fp8 pairs; the resid*resid cross term is dropped, ~0.4% second order).
  - All fp8 matmuls use MatmulPerfMode.DoubleRow (2 contraction subtiles per
    instruction at 0.5 cycles/output-column = 4x bf16 throughput). The
    HD=128 score contraction is split as [64 partitions x 2 subtiles].
  - P*V stays bf16 (p values cluster at 1.0; fp8 would quantize away the
    softmax signal).

Scale bookkeeping (powers of two, folded into existing constants):
  x8,w8 carry 2^7 each -> projection PSUM = 2^14 * true.
  cos2/sin2 carry 2^-8   -> Q^T/K^T fp8 = 2^6 * true; score PSUM = 2^12 * s.
  exp scale = 2^-12/sqrt(HD).  V stays scaled: vts = 2^14 * v.
  ones_mat = 16 = 2^(14-10)  -> ao = 2^10 * attn (good fp8 range).
  out PSUM = 2^(10+7) * true -> final ACT copy uses scale 2^-17.

Softmax denominator: DVE accumulates exp chunks into two bf16 accumulators
(even/odd chunks, so the serial add chain keeps up with the PE), then two
ones-matmuls fold the partition sum + broadcast + 2^4 scale in one step.

Collectives: two AllGathers over sequence halves (all 4 heads + resid rows
per half), issued after qb=1 and qb=3 of the qb-outer attention loop; the
output projection consumes half 0 while half 1 is still being gathered.
"""

import os

import numpy as np
import ml_dtypes

import concourse.bass as bass
import concourse.mybir as mybir
import concourse.tile as tile
from concourse import bacc
from concourse.bass_utils import run_bass_kernel_spmd

# Problem dims (hardcoded per contract)
B, S, D = 1, 2048, 4096
HQ, HK, HD = 32, 8, 128
NCORES = 8
HQL = HQ // NCORES          # 4 local q heads
SB = 512                    # seq block (matmul moving free dim)
NB = S // SB                # 4 seq blocks
NPAIR = D // 256            # 16 DoubleRow contraction pairs for D
SCALE = 1.0 / float(np.sqrt(HD))
EXP_SCALE = SCALE / 4096.0  # scores PSUM carries 2^12
S2 = S // 2                 # gather half width

F32 = mybir.dt.float32
BF16 = mybir.dt.bfloat16
FP8 = mybir.dt.float8e4
DR = mybir.MatmulPerfMode.DoubleRow
DEBUG_DUMPS = os.environ.get("BASSDBG", "") == "1"

# stream_shuffle mask: swap adjacent pairs within each 32-partition quadrant
SWAP_MASK = [(i ^ 1) for i in range(32)]


def _build_nc():
    nc = bacc.Bacc(
        "TRN2", target_bir_lowering=False, debug=False, num_devices=NCORES
    )

    io = {}
    io["x8"] = nc.dram_tensor("x8", [D, S], FP8, kind="ExternalInput")
    io["xbf"] = nc.dram_tensor("xbf", [D, S], BF16, kind="ExternalInput")
    io["wq8"] = nc.dram_tensor("wq8", [D, HQL * HD], FP8, kind="ExternalInput")
    io["wk8"] = nc.dram_tensor("wk8", [D, HD], FP8, kind="ExternalInput")
    io["wv"] = nc.dram_tensor("wv", [D, HD], BF16, kind="ExternalInput")
    io["wo"] = nc.dram_tensor("wo", [D, HQL * HD], BF16, kind="ExternalInput")
    io["cos2"] = nc.dram_tensor("cos2", [HD, S], BF16, kind="ExternalInput")
    io["sin2"] = nc.dram_tensor("sin2", [HD, S], BF16, kind="ExternalInput")
    # causal mask as a matmul: T8 lower-inclusive triangle (stationary) and
    # per-td moving panels R with -1e6 markers; T8.T @ R[td] adds -1e6 to
    # every (p, c) with c < 128*td + p, exactly the invalid region.
    io["mskT"] = nc.dram_tensor("mskT", [128, 128], BF16, kind="ExternalInput")
    io["mskR"] = nc.dram_tensor("mskR", [128, NB, SB], BF16, kind="ExternalInput")
    io["ident"] = nc.dram_tensor("ident", [128, 128], BF16, kind="ExternalInput")
    io["outT"] = nc.dram_tensor("outT", [HQL * HD, S], F32, kind="ExternalOutput")
    if DEBUG_DUMPS:
        io["dbg_qt"] = nc.dram_tensor(
            "dbg_qt", [128, NB, HQL, SB], FP8, kind="ExternalOutput"
        )
        io["dbg_kt"] = nc.dram_tensor(
            "dbg_kt", [128, NB, SB], FP8, kind="ExternalOutput"
        )
        io["dbg_vs"] = nc.dram_tensor(
            "dbg_vs", [128, NB, SB // 128, HD], BF16, kind="ExternalOutput"
        )
        io["dbg_loc"] = nc.dram_tensor(
            "dbg_loc", [HQL * HD, SB], BF16, kind="ExternalOutput"
        )

    with tile.TileContext(nc) as tc:
        _body(tc, io)
    nc.compile()
    return nc


def _body(tc, io):
    nc = tc.nc
    from contextlib import ExitStack

    ctx = ExitStack()
    with ctx:
        consts = ctx.enter_context(tc.tile_pool(name="consts", bufs=1))
        qkv = ctx.enter_context(tc.tile_pool(name="qkv", bufs=1))
        dram = ctx.enter_context(tc.tile_pool(name="dram", bufs=1, space="DRAM"))

        cos2 = consts.tile([HD, S], BF16)
        sin2 = consts.tile([HD, S], BF16)
        ident = consts.tile([128, 128], BF16)
        mskT = consts.tile([128, 128], BF16)
        mskR = consts.tile([128, NB, SB], BF16)
        # ones * 2^14: the denominator matmul folds partition-sum, broadcast
        # and the 2^14 V-path descale in one shot -> ao lands at true scale
        ones_mat = consts.tile([128, 128], BF16)
        nc.vector.memset(ones_mat, 16384.0)

        # persistent per-core tensors, split per s-block for fine-grained deps
        qt8f = [
            qkv.tile([128, HQL, SB], FP8, name=f"qt8f{sb}") for sb in range(NB)
        ]
        kt8f = [qkv.tile([128, SB], FP8, name=f"kt8f{sb}") for sb in range(NB)]
        # [64, 2, ...] split layouts for DoubleRow score matmuls
        qt8 = [
            qkv.tile([64, 2, HQL, SB], FP8, name=f"qt8_{sb}") for sb in range(NB)
        ]
        kt8 = [qkv.tile([64, 2, SB], FP8, name=f"kt8_{sb}") for sb in range(NB)]
        vs_sb = [
            qkv.tile([128, SB // 128, HD], BF16, name=f"vs{sb}") for sb in range(NB)
        ]

        # quarter-sequence bounce + gather buffers (bf16, 4 heads per qb
        # block); gathered row 128*c = contraction chunk c = q-head 4i+j
        attn_loc = [
            dram.tile([HQL * HD, SB], BF16, name=f"attn_loc{g2}")
            for g2 in range(NB)
        ]
        attn_g = [
            dram.tile(
                [NCORES * HQL * HD, SB],
                BF16,
                name=f"attn_g{g2}",
                addr_space="Shared",
            )
            for g2 in range(NB)
        ]

        # ================= Stage A: projections + RoPE =================
        with ctx_pools(tc) as (wpool, xpool, rpool, psA):
            # warm-up DoubleRow (discarded): the first dual-fp8 ldweights in a
            # program mis-executes (partial-NaN psum); absorb it on zeros.
            dmy_l = wpool.tile([128, 2, 128], FP8, name="dmy_l")
            dmy_r = wpool.tile([128, 2, 8], FP8, name="dmy_r")
            nc.vector.memset(dmy_l, 0.0)
            nc.vector.memset(dmy_r, 0.0)
            ps_warm = psA.tile([128, 8], F32, name="ps_warm", tag="psvt", bufs=2)
            nc.tensor.matmul(
                ps_warm, lhsT=dmy_l, rhs=dmy_r, start=True, stop=True, perf_mode=DR
            )

            wq_sb = wpool.tile([128, 2 * NPAIR, HQL * HD], FP8)
            wk_sb = wpool.tile([128, 2 * NPAIR, HD], FP8)
            wv_sb = wpool.tile([128, 2 * NPAIR, HD], BF16)
            # wq first in 4 big chunks (Q groups consume it first), then
            # wk/wv; consts ride the scalar rail which is idle at startup
            for c4 in range(NPAIR // 2):
                sl = slice(c4 * 512, (c4 + 1) * 512)
                nc.gpsimd.dma_start(
                    out=wq_sb[:, c4 * 4 : c4 * 4 + 4, :],
                    in_=io["wq8"][sl, :].rearrange("(c p) n -> p c n", p=128),
                )
            nc.scalar.dma_start(out=ident, in_=io["ident"][:, :])
            nc.scalar.dma_start(out=cos2, in_=io["cos2"][:, :])
            nc.scalar.dma_start(out=sin2, in_=io["sin2"][:, :])
            nc.scalar.dma_start(out=mskT, in_=io["mskT"][:, :])
            nc.scalar.dma_start(out=mskR, in_=io["mskR"][:, :, :])
            for w_sb, nm in ((wk_sb, "wk8"), (wv_sb, "wv")):
                for c4 in range(NPAIR // 2):
                    sl = slice(c4 * 512, (c4 + 1) * 512)
                    nc.gpsimd.dma_start(
                        out=w_sb[:, c4 * 4 : c4 * 4 + 4, :],
                        in_=io[nm][sl, :].rearrange("(c p) n -> p c n", p=128),
                    )

            for sb in range(NB):
                ssl = slice(sb * SB, (sb + 1) * SB)
                ps_q = [
                    psA.tile(
                        [128, SB],
                        F32,
                        name=f"psq{t}_{sb}",
                        tag=f"psq{t}",
                        bufs=1,
                    )
                    for t in range(HQL)
                ]
                ps_k = psA.tile([128, SB], F32, tag="psk")
                ps_v = psA.tile([128, SB], F32, tag="psv")
                # load all 16 pairs up front, then run each output's psum
                # accumulation group back-to-back (single-group marginal cost
                # on the PE is ~20% cheaper than 6-way group interleave)
                # x8 pair quads on sync (Q/K fp8 DR); xbf quads on scalar
                # (V in full bf16 -- no residual streams, fewer instructions)
                x8q, xbq = [], []
                for q4 in range(NPAIR // 4):
                    xq = xpool.tile([128, 8, SB], FP8, name=f"x8q{sb}_{q4}", tag="x8q")
                    rsl = slice(q4 * 1024, (q4 + 1) * 1024)
                    nc.sync.dma_start(
                        out=xq,
                        in_=io["x8"][rsl, ssl].rearrange("(c p) n -> p c n", p=128),
                    )
                    x8q.append(xq)
                    xb = xpool.tile(
                        [128, 8, SB], BF16, name=f"xbq{sb}_{q4}", tag="xbq"
                    )
                    nc.scalar.dma_start(
                        out=xb,
                        in_=io["xbf"][rsl, ssl].rearrange("(c p) n -> p c n", p=128),
                    )
                    xbq.append(xb)

                def x8p(c):
                    return x8q[c // 4][:, (c % 4) * 2 : (c % 4) * 2 + 2, :]

                for t in range(HQL):
                    for c in range(NPAIR):
                        nc.tensor.matmul(
                            ps_q[t],
                            lhsT=wq_sb[:, 2 * c : 2 * c + 2, t * 128 : (t + 1) * 128],
                            rhs=x8p(c),
                            start=c == 0,
                            stop=c == NPAIR - 1,
                            perf_mode=DR,
                        )
                for c in range(NPAIR):
                    nc.tensor.matmul(
                        ps_k, lhsT=wk_sb[:, 2 * c : 2 * c + 2, :], rhs=x8p(c),
                        start=c == 0, stop=c == NPAIR - 1, perf_mode=DR,
                    )
                for c in range(2 * NPAIR):
                    nc.tensor.matmul(
                        ps_v, lhsT=wv_sb[:, c, :], rhs=xbq[c // 8][:, c % 8, :],
                        start=c == 0, stop=c == 2 * NPAIR - 1,
                    )

                # V^T -> V (PE transpose per 128-col chunk)
                vts = rpool.tile([128, SB], BF16, name=f"vts{sb}", tag="vts")
                nc.scalar.copy(vts, ps_v)
                for u in range(SB // 128):
                    ps_vt = psA.tile(
                        [128, 128], BF16, name=f"psvt{sb}_{u}", tag="psvt", bufs=2
                    )
                    nc.tensor.transpose(
                        ps_vt, vts[:, u * 128 : (u + 1) * 128], ident
                    )
                    nc.vector.tensor_copy(vs_sb[sb][:, u, :], ps_vt)

                # RoPE -> fp8: rot(q) = q*cos2 + pairswap(q)*sin2, all bf16
                # muls, fp8 destination. cos2/sin2 carry 2^-8.
                def rope(ps, dst, idx):
                    qc = rpool.tile([128, SB], BF16, name=f"qc{idx}", tag="qc")
                    nc.scalar.copy(qc, ps)
                    sw = rpool.tile([128, SB], BF16, name=f"sw{idx}", tag="sw")
                    nc.vector.stream_shuffle(sw, qc, SWAP_MASK)
                    t1 = rpool.tile([128, SB], BF16, name=f"t1{idx}", tag="t1")
                    nc.vector.tensor_mul(t1, qc, cos2[:, ssl])
                    t2 = rpool.tile([128, SB], BF16, name=f"t2{idx}", tag="t2")
                    nc.vector.tensor_mul(t2, sw, sin2[:, ssl])
                    nc.vector.tensor_add(dst, t1, t2)

                for t in range(HQL):
                    rope(ps_q[t], qt8f[sb][:, t, :], f"q{sb}_{t}")
                rope(ps_k, kt8f[sb], f"k{sb}")

                # split [128, .] -> [64, 2, .] for DoubleRow score matmuls
                for t in range(HQL):
                    nc.gpsimd.dma_start(out=qt8[sb][:, 0, t, :], in_=qt8f[sb][0:64, t, :])
                    nc.gpsimd.dma_start(out=qt8[sb][:, 1, t, :], in_=qt8f[sb][64:128, t, :])
                nc.gpsimd.dma_start(out=kt8[sb][:, 0, :], in_=kt8f[sb][0:64, :])
                nc.gpsimd.dma_start(out=kt8[sb][:, 1, :], in_=kt8f[sb][64:128, :])
                if DEBUG_DUMPS:
                    nc.gpsimd.dma_start(out=io["dbg_qt"][:, sb, :, :], in_=qt8f[sb])
                    nc.gpsimd.dma_start(out=io["dbg_kt"][:, sb, :], in_=kt8f[sb])
                    nc.gpsimd.dma_start(out=io["dbg_vs"][:, sb, :, :], in_=vs_sb[sb])

        # wo loads fill DMA idle time during stage B
        wo_pool = ctx.enter_context(tc.tile_pool(name="wo_pool", bufs=1))
        wo_sb = wo_pool.tile([128, 2 * NPAIR, HQL * HD], BF16)
        for c4 in range(NPAIR // 2):
            sl = slice(c4 * 512, (c4 + 1) * 512)
            nc.gpsimd.dma_start(
                out=wo_sb[:, c4 * 4 : c4 * 4 + 4, :],
                in_=io["wo"][sl, :].rearrange("(c p) n -> p c n", p=128),
            )

        apool = ctx.enter_context(tc.tile_pool(name="apool", bufs=6))
        opool = ctx.enter_context(tc.tile_pool(name="opool", bufs=4))

        # ================= Stage B: attention (qb outer) =================
        # Unit = 2 sk-chunks sharing one 2-bank PSUM tile -> one wide exp.
        # Pipeline: PV of unit u-2 is emitted after exp of unit u so the PE
        # never waits on ACT latency. Causal mask applied on the PE (extra
        # mskT x mskR[td] matmul into the scores PSUM on diagonal chunks).
        # Denominator: DVE and GpSimd each own a bf16 accumulator (even/odd
        # chunk of each unit); two ones(2^14)-matmuls finish it.
        with ctx_pools_b(tc) as (ppool, spool, psB):
            for qb in range(NB):
                for h in range(HQL):
                    nkc = (qb + 1) * (SB // 128)
                    nu = nkc // 2
                    ps_o = psB.tile(
                        [128, SB], F32, name=f"pso{h}_{qb}", tag="pso", bufs=2
                    )
                    ps_n = psB.tile(
                        [128, SB], F32, name=f"psn{h}_{qb}", tag="psn", bufs=2
                    )
                    pts = {}

                    def consume_u(u, last, h=h, qb=qb, ps_o=ps_o, ps_n=ps_n,
                                  pts=pts):
                        pt2 = pts.pop(u)
                        for v in range(2):
                            kc = 2 * u + v
                            nc.tensor.matmul(
                                ps_o,
                                lhsT=vs_sb[kc // 4][:, kc % 4, :],
                                rhs=pt2[:, v, :],
                                start=kc == 0,
                                stop=last and v == 1,
                            )
                            # denominator: ones(2^14)-matmul accumulates the
                            # partition sum of exp, pre-broadcast + descaled
                            nc.tensor.matmul(
                                ps_n,
                                lhsT=ones_mat,
                                rhs=pt2[:, v, :],
                                start=kc == 0,
                                stop=last and v == 1,
                            )

                    for u in range(nu):
                        ps_s = psB.tile(
                            [128, 2, SB], F32, name=f"pss{h}_{qb}_{u}", tag="pss",
                            bufs=2,
                        )
                        for v in range(2):
                            kc = 2 * u + v
                            td = kc - qb * 4
                            nc.tensor.matmul(
                                ps_s[:, v, :],
                                lhsT=kt8[kc // 4][
                                    :, :, (kc % 4) * 128 : (kc % 4 + 1) * 128
                                ],
                                rhs=qt8[qb][:, :, h, :],
                                start=True,
                                stop=td < 0,
                                perf_mode=DR,
                            )
                            if td >= 0:
                                nc.tensor.matmul(
                                    ps_s[:, v, :],
                                    lhsT=mskT,
                                    rhs=mskR[:, td, :],
                                    start=False,
                                    stop=True,
                                )
                        pt2 = ppool.tile(
                            [128, 2, SB], BF16, name=f"pt{h}_{qb}_{u}", tag="pt"
                        )
                        nc.scalar.activation(
                            pt2, ps_s, mybir.ActivationFunctionType.Exp,
                            scale=EXP_SCALE,
                        )
                        pts[u] = pt2
                        if u >= 2:
                            consume_u(u - 2, last=False)
                    for u in range(max(0, nu - 2), nu):
                        consume_u(u, last=u == nu - 1)

                    rb = spool.tile([128, SB], F32, name=f"rb{h}_{qb}", tag="rb")
                    nc.vector.reciprocal_approx_fast(rb, ps_n)
                    ao = spool.tile(
                        [128, SB], BF16, name=f"ao{h}_{qb}", tag="ao", bufs=4
                    )
                    nc.vector.tensor_mul(ao, ps_o, rb)
                    # sync queue: the gpsimd queue is busy with collectives,
                    # which would backpressure ao -> DVE -> PSUM -> PE
                    nc.sync.dma_start(
                        out=attn_loc[qb][h * 128 : (h + 1) * 128, :], in_=ao
                    )
                if DEBUG_DUMPS and qb == 0:
                    nc.gpsimd.dma_start(
                        out=io["dbg_loc"][:, :], in_=attn_loc[0][:, :]
                    )
                nc.gpsimd.collective_compute(
                    "AllGather",
                    mybir.AluOpType.bypass,
                    replica_groups=[list(range(NCORES))],
                    ins=[attn_loc[qb].opt()],
                    outs=[attn_g[qb].opt()],
                )

        # ============ Stage D: out = attn @ wo (bf16, column shard) ==========
        # Gathered row 128*c of half g2 = contraction chunk c (= q-head 4i+j
        # of core i); wo chunk c rows match. Plain bf16 matmuls, 4 outputs.
        # tile_wait_until: keep the scheduler from hoisting these gather-
        # dependent loads into the stage-B queue regions (an unsatisfied DMA
        # at a queue head blocks every instruction behind it)
        with tc.tile_pool(name="psD", bufs=2, space="PSUM") as psD:
            for g in range(NB):
                osl = slice(g * SB, (g + 1) * SB)
                ats = []
                # staggered scheduler hints: prefetch each g's loads shortly
                # after its gather lands, without blocking stage-B queues
                with tc.tile_wait_until(0.22 + 0.04 * g):
                    for q in range(4):
                        at = apool.tile(
                            [128, 8, SB], BF16, name=f"at{g}_{q}", tag="at"
                        )
                        eng = nc.sync if q % 2 == 0 else nc.scalar
                        eng.dma_start(
                            out=at,
                            in_=attn_g[g][q * 1024 : (q + 1) * 1024, :].rearrange(
                                "(c p) n -> p c n", p=128
                            ),
                        )
                        ats.append(at)
                for n in range(HQL):
                    nsl = slice(n * 128, (n + 1) * 128)
                    ps_d = psD.tile([128, SB], F32, name=f"psd{g}_{n}", tag="psd")
                    for c in range(2 * NPAIR):
                        nc.tensor.matmul(
                            ps_d, lhsT=wo_sb[:, c, nsl], rhs=ats[c // 8][:, c % 8, :],
                            start=c == 0, stop=c == 2 * NPAIR - 1,
                        )
                    ot = opool.tile([128, SB], F32, name=f"ot{g}_{n}", tag="ot")
                    nc.scalar.copy(ot, ps_d)
                    nc.scalar.dma_start(
                        out=io["outT"][n * 128 : (n + 1) * 128, osl], in_=ot
                    )


from contextlib import contextmanager


@contextmanager
def ctx_pools(tc):
    with (
        tc.tile_pool(name="wpool", bufs=1) as wpool,
        tc.tile_pool(name="xpool", bufs=6) as xpool,
        tc.tile_pool(name="rpool", bufs=3) as rpool,
        tc.tile_pool(name="psA", bufs=1, space="PSUM") as psA,
    ):
        yield wpool, xpool, rpool, psA


@contextmanager
def ctx_pools_b(tc):
    with (
        tc.tile_pool(name="ppool", bufs=8) as ppool,
        tc.tile_pool(name="spool", bufs=2) as spool,
        tc.tile_pool(name="psB", bufs=2, space="PSUM") as psB,
    ):
        yield ppool, spool, psB


_NC_CACHE = None


def _get_nc():
    global _NC_CACHE
    if _NC_CACHE is None:
        _NC_CACHE = _build_nc()
    return _NC_CACHE


def _prep_in_maps(x, freqs_cos, freqs_sin, wq, wk, wv, wo):
    bf = ml_dtypes.bfloat16
    f8 = ml_dtypes.float8_e4m3
    S7 = 128.0

    x = np.asarray(x, np.float32).reshape(S, D)
    xT = np.ascontiguousarray(x.T) * S7
    x8 = xT.astype(f8)
    xbf = xT.astype(bf)

    cos = np.asarray(freqs_cos, np.float32)  # [S, HD/2]
    sin = np.asarray(freqs_sin, np.float32)
    cos2 = np.repeat(cos.T, 2, axis=0)  # [HD, S]
    sin_t = sin.T
    sin2 = np.empty((HD, S), np.float32)
    sin2[0::2] = -sin_t
    sin2[1::2] = sin_t
    rs = 1.0 / 256.0  # 2^-8: descale 2^-14, rescale 2^6 for fp8 q/k
    cos2 = cos2 * rs
    sin2 = sin2 * rs

    # causal mask as matmul: mskT.T @ mskR[td] = -1e6 where c < 128*td + p
    kk = np.arange(128)
    mskT = (kk[:, None] <= kk[None, :]).astype(bf)  # [k, p] lower-inclusive
    mskR = np.zeros((128, NB, SB), np.float32)
    for td in range(NB):
        cc = np.arange(SB)
        hit = (cc[None, :] == 128 * td + kk[:, None] - 1).astype(np.float32)
        hit[0, :] += (cc < 128 * td).astype(np.float32)
        mskR[:, td, :] = -1e6 * hit
    ident = np.eye(128, dtype=bf)

    wq = np.asarray(wq, np.float32) * S7
    wk = np.asarray(wk, np.float32) * S7
    wv = np.asarray(wv, np.float32) * S7
    wo = np.asarray(wo, np.float32)
    in_maps = []
    for i in range(NCORES):
        wq_i = np.ascontiguousarray(wq[:, i * HQL * HD : (i + 1) * HQL * HD])
        wk_i = np.ascontiguousarray(wk[:, i * HD : (i + 1) * HD])
        wv_i = np.ascontiguousarray(wv[:, i * HD : (i + 1) * HD])
        wo_i = np.ascontiguousarray(wo[:, i * HQL * HD : (i + 1) * HQL * HD])
        in_maps.append(
            {
                "x8": x8,
                "xbf": xbf,
                "cos2": cos2.astype(bf),
                "sin2": sin2.astype(bf),
                "mskT": mskT,
                "mskR": mskR.astype(bf),
                "ident": ident,
                "wq8": wq_i.astype(f8),
                "wk8": wk_i.astype(f8),
                "wv": wv_i.astype(bf),
                "wo": wo_i.astype(bf),
            }
        )
    return in_maps


def _install_trace_shims():
    """The container's antenv lacks axon_hooks; replicate trn_boot's ctypes
    NTFF hook so run_bass_kernel_spmd(trace=True) works. Also stub out the
    fish-bucket artifact upload (no bucket access here)."""
    import sys
    import types
    import ctypes
    import contextlib

    if "antenv.axon_hooks" not in sys.modules:
        mod = types.ModuleType("antenv.axon_hooks")
        mod._hook = None

        def set_axon_ntff_profile_hook(h):
            mod._hook = h

        def get_axon_ntff_profile_hook():
            return mod._hook

        mod.set_axon_ntff_profile_hook = set_axon_ntff_profile_hook
        mod.get_axon_ntff_profile_hook = get_axon_ntff_profile_hook
        sys.modules["antenv.axon_hooks"] = mod

        so_path = "/opt/axon/libaxon_pjrt.so"
        lib = ctypes.CDLL(so_path)
        if hasattr(lib, "axon_start_nrt_profile"):
            lib.axon_start_nrt_profile.argtypes = [
                ctypes.POINTER(ctypes.c_int64),
                ctypes.c_size_t,
            ]
            lib.axon_start_nrt_profile.restype = ctypes.c_int64
            lib.axon_stop_nrt_profile.argtypes = [ctypes.c_char_p]
            lib.axon_stop_nrt_profile.restype = ctypes.c_int64

            @contextlib.contextmanager
            def _hook(output_dir, device_ids):
                import jax

                jax.devices()
                if device_ids:
                    ids = (ctypes.c_int64 * len(device_ids))(*device_ids)
                    rc = lib.axon_start_nrt_profile(ids, len(device_ids))
                else:
                    rc = lib.axon_start_nrt_profile(None, 0)
                if rc != 0:
                    raise RuntimeError(f"axon_start_nrt_profile rc={rc}")
                try:
                    yield
                finally:
                    n = lib.axon_stop_nrt_profile(str(output_dir).encode())
                    if n <= 0:
                        print(f"WARNING: axon_stop_nrt_profile rc={n}")

            set_axon_ntff_profile_hook(_hook)

    import concourse.bass_utils as bu

    bu.upload_artifacts = lambda tmpdir: "local://" + str(tmpdir)


def run(inputs, trace=False, **kw):
    nc = _get_nc()
    if trace:
        _install_trace_shims()
    in_maps = _prep_in_maps(**inputs)
    res = run_bass_kernel_spmd(nc, in_maps, list(range(NCORES)), trace=trace, **kw)
    out = np.concatenate(
        [res.results[i]["outT"].T for i in range(NCORES)], axis=1
    )
    return out.reshape(B, S, D).astype(np.float32), res


def kernel(x, freqs_cos, freqs_sin, wq, wk, wv, wo):
    out, _ = run(
        dict(
            x=x,
            freqs_cos=freqs_cos,
            freqs_sin=freqs_sin,
            wq=wq,
            wk=wk,
            wv=wv,
            wo=wo,
        )
    )
    return out
